# revision 41
# baseline (speedup 1.0000x reference)
"""Trainium2 Bass kernel for AdaptiveLinearPoolNewsvendorLayer.

Computes, for B=4096 samples across 8 NeuronCores (data parallel, 512/core):
  w        = softmax(MLP(x))                  # [B, 8] mixture weights
  pdf      = sum_e w[:, e] * p_e              # [B, 101] convex combination
  z        = argmin_z of the per-sample newsvendor QP (exact, convex)
  error    = SUPPORT - z

The QP solve is reformulated analytically: with TAU=RA=0.5, EPS=1e-4 the
piecewise-quadratic objective's stationary point on interval k is
  z_unc_k = (Sq + 0.25*P - 0.5*C_k) / Q
with C_k the pdf mass strictly below the interval, P = sum(pdf),
Sq = sum((pdf+EPS)*s), Q = P + 101*EPS.  Since z_unc is non-increasing in k
while the intervals increase, the unique minimizer is
  z = clip(max_k min(z_unc_k, upper_k), 0, 1)
which needs only a cumsum + one fused min/max-reduce per sample instead of
the brute-force 102x101 objective evaluation.
"""

import numpy as np

B, D, H, E, N = 4096, 64, 128, 8, 101
NCORES = 8
BL = B // NCORES          # 512 rows per core
P128 = 128
NT = BL // P128           # 4 partition-tiles per core
EPS = 1e-4
USE_TTR = False

# constant-block column offsets (one [128, CST_COLS] f32 DRAM tensor per core;
# split into a critical DMA [0,_SB_C) = weights+biases+x and a tail DMA)
_W1_C = 0                  # cols [0,128)  W1 [64,128] lhsT, duplicated rows 64:128
_W2_C = 128                # cols [128,256)            : W2 [128,128]
_W3_C = 256                # cols [256,264)            : W3 [128,8]
_B1_C = 264                # col 264                   : b1 [128]
_B2_C = 265                # col 265                   : b2 [128]
_B3_C = 266                # col 266 rows 0:8          : b3 [8]
_X_C = 267                 # cols [267,779) rows 0:64  : xT [64, 512]
_SB_C = 779                # cols [779,880)            : support grid, replicated
_UP_C = 880                # cols [880,981)            : upper_k (k=1..101), replicated
_ID_C = 981                # cols [981,1109)           : I_128
CST_COLS = 1109

_CACHE = {}


def _const_block(W1, b1, W2, b2, W3, b3):
    cst = np.zeros((P128, CST_COLS), np.float32)
    cst[:D, _W1_C:_W1_C + H] = W1
    cst[D:, _W1_C:_W1_C + H] = W1
    cst[:, _W2_C:_W2_C + H] = W2
    cst[:, _W3_C:_W3_C + E] = W3
    cst[:, _B1_C] = b1
    cst[:, _B2_C] = b2
    cst[:E, _B3_C] = b3
    S = np.linspace(0.0, 1.0, N, dtype=np.float32)
    cst[:, _SB_C:_SB_C + N] = S[None, :]
    upper = np.concatenate([S[1:], S[-1:]])
    cst[:, _UP_C:_UP_C + N] = upper[None, :]
    cst[:, _ID_C:_ID_C + P128] = np.eye(P128, dtype=np.float32)
    return cst


def _pack_x(cst, x_core):
    # xT [64, 512] packed as two column-halves stacked: rows 0:64 hold
    # xT[:, 0:256], rows 64:128 hold xT[:, 256:512] (zero waste, and W1 is
    # duplicated into rows 64:128 so layer 1 runs as two matmuls).
    out = cst.copy()
    out[:D, _X_C:_X_C + BL] = x_core.T
    return out


def _build_nc():
    import concourse.bass as bass
    import concourse.bacc as bacc
    import concourse.tile as tile
    from concourse import mybir

    f32 = mybir.dt.float32
    f32r = mybir.dt.float32r
    Alu = mybir.AluOpType
    Act = mybir.ActivationFunctionType
    AxX = mybir.AxisListType.X

    nc = bacc.Bacc()

    p_d = [nc.declare_dram_parameter(f"p{e}", [BL, N], f32, isOutput=False)
           for e in range(E)]
    cst_d = nc.declare_dram_parameter("cst", [P128, CST_COLS], f32,
                                      isOutput=False)
    opdf_d = nc.declare_dram_parameter("out_pdf", [BL, N], f32, isOutput=True)
    oz_d = nc.declare_dram_parameter("out_z", [BL, 1], f32, isOutput=True)
    oerr_d = nc.declare_dram_parameter("out_err", [BL, N], f32, isOutput=True)

    NDV = 4   # experts combined on DVE; the rest go to GpSimd

    with tile.TileContext(nc) as tc:
        with (
            tc.tile_pool(name="consts", bufs=1) as cpool,
            tc.tile_pool(name="mlp", bufs=1) as mpool,
            tc.tile_pool(name="pin", bufs=1) as ppool,
            tc.tile_pool(name="work", bufs=1) as wpool,
            tc.tile_pool(name="acc", bufs=2) as apool,
            tc.tile_pool(name="stats", bufs=1) as spool,
            tc.tile_pool(name="ps_mm", bufs=2, space="PSUM") as psMM,
            tc.tile_pool(name="ps_wu", bufs=1, space="PSUM") as psWU,
            tc.tile_pool(name="ps_wm", bufs=1, space="PSUM") as psWM,
        ):
            # ---- PE p-state warmup: harmless matmuls on a zeroed tile while
            # the input DMAs are in flight (PE ramps to full clock after ~3us
            # of continuous work) ----
            zw = cpool.tile([P128, P128], f32, tag="zw")
            nc.vector.memset(zw[:], 0.0)
            wps = psWM.tile([P128, P128], f32, tag="wps")
            bf16 = mybir.dt.bfloat16
            zwb = zw[:].bitcast(bf16)[:, :P128]
            for _ in range(16):
                nc.tensor.matmul(wps[:], zwb, zwb, start=True, stop=True)
            zwa = cpool.tile([P128, P128], f32, tag="zwa")
            for _ in range(4):
                nc.scalar.activation(zwa[:], zw[:].bitcast(f32), Act.Exp,
                                     bias=0.0, scale=1.0)

            cstr = cpool.tile([P128, CST_COLS], f32r, tag="cst")
            nc.sync.dma_start(out=cstr[:, :_SB_C],
                              in_=cst_d[:, :_SB_C].bitcast(f32r))
            nc.sync.dma_start(out=cstr[:, _SB_C:],
                              in_=cst_d[:, _SB_C:].bitcast(f32r))
            cst = cstr[:].bitcast(f32)
            W1lo = cstr[:D, _W1_C:_W1_C + H]
            W2 = cstr[:, _W2_C:_W2_C + H]
            W3 = cstr[:, _W3_C:_W3_C + E]
            xT = cstr[:D, _X_C:_X_C + BL]
            b1 = cst[:, _B1_C:_B1_C + 1]
            b2 = cst[:, _B2_C:_B2_C + 1]
            b3 = cst[:E, _B3_C:_B3_C + 1]
            sB = cst[:, _SB_C:_SB_C + N]
            upB = cst[:, _UP_C:_UP_C + N]
            idn = cst[:, _ID_C:_ID_C + P128]

            p_sb = []
            for e in range(E):
                pt = ppool.tile([P128, NT * N], f32, tag=f"p{e}")
                nc.sync.dma_start(
                    out=pt[:].rearrange("p (t n) -> p t n", t=NT),
                    in_=p_d[e].rearrange("(p t) n -> p t n", p=P128),
                )
                p_sb.append(pt)

            # ---- MLP in [feature, batch] layout, pipelined in two
            # 256-column chunks (separate PSUM banks so ACT overlaps PE) ----
            HB = BL // 2
            h1 = mpool.tile([P128, BL], f32r, tag="h1")
            h2 = mpool.tile([P128, BL], f32r, tag="h2")
            wu = mpool.tile([E, BL], f32, tag="wu")
            for c in range(2):
                cs = slice(c * HB, (c + 1) * HB)
                h1p = psMM.tile([P128, HB], f32, tag="mm1")
                nc.tensor.matmul(h1p[:], W1lo, xT[:, cs], start=True, stop=True)
                nc.scalar.activation(h1[:, cs], h1p[:], Act.Relu, bias=b1,
                                     scale=1.0)
                h2p = psMM.tile([P128, HB], f32, tag="mm2")
                nc.tensor.matmul(h2p[:], W2, h1[:, cs], start=True, stop=True)
                nc.scalar.activation(h2[:, cs], h2p[:], Act.Relu, bias=b2,
                                     scale=1.0)
                lgp = psMM.tile([E, HB], f32, tag="mm3")
                nc.tensor.matmul(lgp[:], W3, h2[:, cs], start=True, stop=True)
                nc.scalar.activation(wu[:, cs], lgp[:], Act.Exp, bias=b3,
                                     scale=1.0)

            # ---- mixture weights back to [batch, expert] layout.
            # Sample b lives at (partition, tile) = (b // 4, b % 4), so tile t
            # takes the stride-4 column slice of wu. ----
            wuv = wu[:].rearrange("e (p t) -> e t p", t=NT)
            wuT = psWU.tile([P128, NT * E], f32, tag="wuT")
            for t in range(NT):
                nc.tensor.transpose(
                    wuT[:, t * E:(t + 1) * E], wuv[:, t, :], idn[:E, :E],
                )
            se4 = spool.tile([P128, NT], f32, tag="se4")
            nc.vector.tensor_reduce(
                se4[:], wuT[:].rearrange("p (t e) -> p t e", t=NT), axis=AxX,
                op=Alu.add,
            )
            r4 = spool.tile([P128, NT], f32, tag="r4")
            nc.vector.reciprocal(r4[:], se4[:])
            wn = spool.tile([P128, NT * E], f32, tag="wn")
            nc.vector.tensor_tensor(
                out=wn[:].rearrange("p (t e) -> p t e", t=NT),
                in0=wuT[:].rearrange("p (t e) -> p t e", t=NT),
                in1=r4[:].unsqueeze(2).broadcast_to([P128, NT, E]),
                op=Alu.mult,
            )

            # ---- combine: per-expert broadcast-TT over all 4 tiles at once
            # (DVE experts 0-4, GpSimd 5-7; Pool has no TensorScalarPtr) ----
            SC4 = spool.tile([P128, NT], f32, tag="SC4")
            comb_all = wpool.tile([P128, NT * N], f32, tag="comb")
            ci_all = wpool.tile([P128, NT * N], f32, tag="ci")
            wnv = wn[:].rearrange("p (t e) -> p t e", t=NT)

            def wb(e):
                return wnv[:, :, e].unsqueeze(2).broadcast_to([P128, NT, N])

            def p3(e):
                return p_sb[e][:].rearrange("p (t n) -> p t n", t=NT)

            bf16 = mybir.dt.bfloat16
            yv = [apool.tile([P128, NT * N], bf16, name=f"y{e}", tag=f"y{e}")
                  for e in range(E)]
            for e in range(4):
                nc.vector.tensor_tensor(
                    yv[e][:].rearrange("p (t n) -> p t n", t=NT), p3(e), wb(e),
                    op=Alu.mult)
            # experts 4,5 multiply on GpSimd: its TT runs while DVE does
            # broadcast mults (no 2-input DVE ops yet, so no port contention)
            for e in (4, 5):
                nc.gpsimd.tensor_tensor(
                    yv[e][:].rearrange("p (t n) -> p t n", t=NT), p3(e), wb(e),
                    op=Alu.mult)
            # experts 6,7 multiply on the otherwise-idle ScalarE (per-tile
            # Copy with per-partition scale)
            for e in (6, 7):
                for t in range(NT):
                    sl = slice(t * N, (t + 1) * N)
                    nc.scalar.activation(
                        yv[e][:, sl], p_sb[e][:, sl], Act.Copy,
                        bias=0.0, scale=wn[:, t * E + e:t * E + e + 1],
                    )
            s01 = apool.tile([P128, NT * N], bf16, tag="s01")
            s23 = apool.tile([P128, NT * N], bf16, tag="s23")
            s45 = apool.tile([P128, NT * N], bf16, tag="s45")
            s67 = apool.tile([P128, NT * N], bf16, tag="s67")
            nc.vector.tensor_tensor(s01[:], yv[0][:], yv[1][:], op=Alu.add)
            nc.vector.tensor_tensor(s23[:], yv[2][:], yv[3][:], op=Alu.add)
            nc.vector.tensor_tensor(s45[:], yv[4][:], yv[5][:], op=Alu.add)
            nc.vector.tensor_tensor(s67[:], yv[6][:], yv[7][:], op=Alu.add)
            s03 = apool.tile([P128, NT * N], bf16, tag="s03")
            s47 = apool.tile([P128, NT * N], bf16, tag="s47")
            nc.vector.tensor_tensor(s03[:], s01[:], s23[:], op=Alu.add)
            nc.vector.tensor_tensor(s47[:], s45[:], s67[:], op=Alu.add)
            for t in range(NT):
                sl = slice(t * N, (t + 1) * N)
                nc.vector.tensor_tensor_scan(
                    out=ci_all[:, sl], data0=s03[:, sl],
                    data1=s47[:, sl], initial=0.0,
                    op0=Alu.add, op1=Alu.add,
                )
                nc.vector.tensor_reduce(
                    SC4[:, t:t + 1], ci_all[:, sl], axis=AxX, op=Alu.add)
            nc.vector.tensor_tensor(comb_all[:], s03[:], s47[:], op=Alu.add)
            nc.sync.dma_start(
                out=opdf_d.rearrange("(p t) n -> p t n", p=P128),
                in_=comb_all[:].rearrange("p (t n) -> p t n", t=NT),
            )

            # ---- batched per-sample scalars ----
            # S1 = 1.01*P - 0.01*SC ; Sq = S1 + EPS*sum(s) ; n0 = Sq + 0.25*P
            #    = 1.26*P - 0.01*SC + 0.00505 ; Q = P + 101*EPS
            P4 = ci_all[:].rearrange("p (t n) -> p t n", t=NT)[:, :, N - 1]
            a4 = spool.tile([P128, NT], f32, tag="a4")
            nc.vector.tensor_scalar(a4[:], SC4[:], -0.01, 0.00505, Alu.mult,
                                    Alu.add)
            n04 = spool.tile([P128, NT], f32, tag="n04")
            nc.vector.scalar_tensor_tensor(
                out=n04[:], in0=P4[:], scalar=1.26, in1=a4[:],
                op0=Alu.mult, op1=Alu.add,
            )
            Q4 = spool.tile([P128, NT], f32, tag="Q4")
            nc.vector.tensor_scalar(Q4[:], P4[:], 101.0 * EPS, None, Alu.add)
            Qr4 = spool.tile([P128, NT], f32, tag="Qr4")
            nc.vector.reciprocal(Qr4[:], Q4[:])
            scl4 = spool.tile([P128, NT], f32, tag="scl4")
            nc.vector.tensor_scalar(scl4[:], Qr4[:], -0.5, None, Alu.mult)
            bias4 = spool.tile([P128, NT], f32, tag="bias4")
            nc.vector.tensor_tensor(bias4[:], n04[:], Qr4[:], op=Alu.mult)

            # ---- z = clip(max_k min(z_unc_k, upper_k), 0, 1) per tile ----
            z4c = spool.tile([P128, NT], f32, tag="z4c")
            zu_all = wpool.tile([P128, NT * N], f32, tag="zu")
            N1 = N + 1
            scr_all = wpool.tile([P128, NT * N1], f32, tag="scr")
            nc.vector.memset(scr_all[:], 0.0)
            for t in range(NT):
                sl = slice(t * N, (t + 1) * N)
                nc.scalar.activation(
                    zu_all[:, sl], ci_all[:, sl], Act.Identity,
                    bias=bias4[:, t:t + 1], scale=scl4[:, t:t + 1],
                )
                nc.vector.tensor_tensor(
                    scr_all[:, t * N1:t * N1 + N], zu_all[:, sl], upB,
                    op=Alu.min)
                # the zeroed extra column folds the z >= 0 clip into the max
                nc.vector.tensor_reduce(
                    z4c[:, t:t + 1], scr_all[:, t * N1:(t + 1) * N1],
                    axis=AxX, op=Alu.max)
            nc.sync.dma_start(
                out=oz_d.rearrange("(p t) one -> p t one", p=P128),
                in_=z4c[:].unsqueeze(2),
            )
            err_all = wpool.tile([P128, NT * N], f32, tag="err")
            HT = NT // 2
            oerr_v = oerr_d.rearrange("(p h t) n -> p h t n", p=P128, h=2)
            for hh in range(2):
                hsl = slice(hh * HT * N, (hh + 1) * HT * N)
                nc.vector.tensor_tensor(
                    err_all[:, hsl].rearrange("p (t n) -> p t n", t=HT),
                    sB.unsqueeze(1).broadcast_to([P128, HT, N]),
                    z4c[:, hh * HT:(hh + 1) * HT].unsqueeze(2)
                        .broadcast_to([P128, HT, N]),
                    op=Alu.subtract)
                nc.sync.dma_start(
                    out=oerr_v[:, hh, :, :],
                    in_=err_all[:, hsl].rearrange("p (t n) -> p t n", t=HT),
                )
    nc.compile()
    return nc


def _get_nc():
    if "nc" not in _CACHE:
        _CACHE["nc"] = _build_nc()
    return _CACHE["nc"]


def _run(x, p0, p1, p2, p3, p4, p5, p6, p7, W1, b1, W2, b2, W3, b3, **runkw):
    from concourse.bass_utils import run_bass_kernel_spmd

    nc = _get_nc()
    cst = _const_block(
        np.asarray(W1, np.float32), np.asarray(b1, np.float32),
        np.asarray(W2, np.float32), np.asarray(b2, np.float32),
        np.asarray(W3, np.float32), np.asarray(b3, np.float32),
    )
    ps = [np.ascontiguousarray(np.asarray(p, np.float32))
          for p in (p0, p1, p2, p3, p4, p5, p6, p7)]
    x = np.ascontiguousarray(np.asarray(x, np.float32))
    in_maps = []
    for c in range(NCORES):
        sl = slice(c * BL, (c + 1) * BL)
        m = {"cst": _pack_x(cst, x[sl])}
        for e in range(E):
            m[f"p{e}"] = ps[e][sl]
        in_maps.append(m)
    res = run_bass_kernel_spmd(nc, in_maps, core_ids=list(range(NCORES)), **runkw)
    results = res.results
    pdf = np.concatenate([results[c]["out_pdf"] for c in range(NCORES)], axis=0)
    z = np.concatenate([results[c]["out_z"] for c in range(NCORES)], axis=0)
    err = np.concatenate([results[c]["out_err"] for c in range(NCORES)], axis=0)
    return (pdf, z, err), res


def kernel(x, p0, p1, p2, p3, p4, p5, p6, p7, W1, b1, W2, b2, W3, b3):
    out, _ = _run(x, p0, p1, p2, p3, p4, p5, p6, p7,
                  W1, b1, W2, b2, W3, b3)
    return out


# revision 42
# speedup vs baseline: 1.0301x; 1.0301x over previous
"""Trainium2 Bass kernel for AdaptiveLinearPoolNewsvendorLayer.

Computes, for B=4096 samples across 8 NeuronCores (data parallel, 512/core):
  w        = softmax(MLP(x))                  # [B, 8] mixture weights
  pdf      = sum_e w[:, e] * p_e              # [B, 101] convex combination
  z        = argmin_z of the per-sample newsvendor QP (exact, convex)
  error    = SUPPORT - z

The QP solve is reformulated analytically: with TAU=RA=0.5, EPS=1e-4 the
piecewise-quadratic objective's stationary point on interval k is
  z_unc_k = (Sq + 0.25*P - 0.5*C_k) / Q
with C_k the pdf mass strictly below the interval, P = sum(pdf),
Sq = sum((pdf+EPS)*s), Q = P + 101*EPS.  Since z_unc is non-increasing in k
while the intervals increase, the unique minimizer is
  z = clip(max_k min(z_unc_k, upper_k), 0, 1)
which needs only a cumsum + one fused min/max-reduce per sample instead of
the brute-force 102x101 objective evaluation.
"""

import numpy as np

B, D, H, E, N = 4096, 64, 128, 8, 101
NCORES = 8
BL = B // NCORES          # 512 rows per core
P128 = 128
NT = BL // P128           # 4 partition-tiles per core
EPS = 1e-4
USE_TTR = False

# constant-block column offsets (one [128, CST_COLS] f32 DRAM tensor per core;
# split into a critical DMA [0,_SB_C) = weights+biases+x and a tail DMA)
_W1_C = 0                  # cols [0,128)  W1 [64,128] lhsT, duplicated rows 64:128
_W2_C = 128                # cols [128,256)            : W2 [128,128]
_W3_C = 256                # cols [256,264)            : W3 [128,8]
_B1_C = 264                # col 264                   : b1 [128]
_B2_C = 265                # col 265                   : b2 [128]
_B3_C = 266                # col 266 rows 0:8          : b3 [8]
_X_C = 267                 # cols [267,779) rows 0:64  : xT [64, 512]
_SB_C = 779                # cols [779,880)            : support grid, replicated
_UP_C = 880                # cols [880,981)            : upper_k (k=1..101), replicated
_ID_C = 981                # cols [981,1109)           : I_128
CST_COLS = 1109

_CACHE = {}


def _const_block(W1, b1, W2, b2, W3, b3):
    cst = np.zeros((P128, CST_COLS), np.float32)
    cst[:D, _W1_C:_W1_C + H] = W1
    cst[D:, _W1_C:_W1_C + H] = W1
    cst[:, _W2_C:_W2_C + H] = W2
    cst[:, _W3_C:_W3_C + E] = W3
    cst[:, _B1_C] = b1
    cst[:, _B2_C] = b2
    cst[:E, _B3_C] = b3
    S = np.linspace(0.0, 1.0, N, dtype=np.float32)
    cst[:, _SB_C:_SB_C + N] = S[None, :]
    upper = np.concatenate([S[1:], S[-1:]])
    cst[:, _UP_C:_UP_C + N] = upper[None, :]
    cst[:, _ID_C:_ID_C + P128] = np.eye(P128, dtype=np.float32)
    return cst


def _pack_x(cst, x_core):
    # xT [64, 512] packed as two column-halves stacked: rows 0:64 hold
    # xT[:, 0:256], rows 64:128 hold xT[:, 256:512] (zero waste, and W1 is
    # duplicated into rows 64:128 so layer 1 runs as two matmuls).
    out = cst.copy()
    out[:D, _X_C:_X_C + BL] = x_core.T
    return out


def _build_nc():
    import concourse.bass as bass
    import concourse.bacc as bacc
    import concourse.tile as tile
    from concourse import mybir

    f32 = mybir.dt.float32
    f32r = mybir.dt.float32r
    Alu = mybir.AluOpType
    Act = mybir.ActivationFunctionType
    AxX = mybir.AxisListType.X

    nc = bacc.Bacc()

    p_d = [nc.declare_dram_parameter(f"p{e}", [BL, N], f32, isOutput=False)
           for e in range(E)]
    cst_d = nc.declare_dram_parameter("cst", [P128, CST_COLS], f32,
                                      isOutput=False)
    opdf_d = nc.declare_dram_parameter("out_pdf", [BL, N], f32, isOutput=True)
    oz_d = nc.declare_dram_parameter("out_z", [BL, 1], f32, isOutput=True)
    oerr_d = nc.declare_dram_parameter("out_err", [BL, N], f32, isOutput=True)

    NDV = 4   # experts combined on DVE; the rest go to GpSimd

    with tile.TileContext(nc) as tc:
        with (
            tc.tile_pool(name="consts", bufs=1) as cpool,
            tc.tile_pool(name="mlp", bufs=1) as mpool,
            tc.tile_pool(name="pin", bufs=1) as ppool,
            tc.tile_pool(name="work", bufs=1) as wpool,
            tc.tile_pool(name="acc", bufs=2) as apool,
            tc.tile_pool(name="stats", bufs=1) as spool,
            tc.tile_pool(name="ps_mm", bufs=2, space="PSUM") as psMM,
            tc.tile_pool(name="ps_wu", bufs=1, space="PSUM") as psWU,
            tc.tile_pool(name="ps_wm", bufs=1, space="PSUM") as psWM,
        ):
            # ---- PE p-state warmup: harmless matmuls on a zeroed tile while
            # the input DMAs are in flight (PE ramps to full clock after ~3us
            # of continuous work) ----
            zw = cpool.tile([P128, P128], f32, tag="zw")
            nc.vector.memset(zw[:], 0.0)
            wps = psWM.tile([P128, P128], f32, tag="wps")
            bf16 = mybir.dt.bfloat16
            zwb = zw[:].bitcast(bf16)[:, :P128]
            for _ in range(16):
                nc.tensor.matmul(wps[:], zwb, zwb, start=True, stop=True)
            zwa = cpool.tile([P128, P128], f32, tag="zwa")
            for _ in range(4):
                nc.scalar.activation(zwa[:], zw[:].bitcast(f32), Act.Exp,
                                     bias=0.0, scale=1.0)

            cstr = cpool.tile([P128, CST_COLS], f32r, tag="cst")
            nc.sync.dma_start(out=cstr[:, :_SB_C],
                              in_=cst_d[:, :_SB_C].bitcast(f32r))
            nc.sync.dma_start(out=cstr[:, _SB_C:],
                              in_=cst_d[:, _SB_C:].bitcast(f32r))
            cst = cstr[:].bitcast(f32)
            W1lo = cstr[:D, _W1_C:_W1_C + H]
            W2 = cstr[:, _W2_C:_W2_C + H]
            W3 = cstr[:, _W3_C:_W3_C + E]
            xT = cstr[:D, _X_C:_X_C + BL]
            b1 = cst[:, _B1_C:_B1_C + 1]
            b2 = cst[:, _B2_C:_B2_C + 1]
            b3 = cst[:E, _B3_C:_B3_C + 1]
            sB = cst[:, _SB_C:_SB_C + N]
            upB = cst[:, _UP_C:_UP_C + N]
            idn = cst[:, _ID_C:_ID_C + P128]

            p_sb = []
            for e in range(E):
                pt = ppool.tile([P128, NT * N], f32, tag=f"p{e}")
                nc.sync.dma_start(
                    out=pt[:].rearrange("p (t n) -> p t n", t=NT),
                    in_=p_d[e].rearrange("(p t) n -> p t n", p=P128),
                )
                p_sb.append(pt)

            # ---- MLP in [feature, batch] layout, pipelined in two
            # 256-column chunks (separate PSUM banks so ACT overlaps PE) ----
            HB = BL // 2
            h1 = mpool.tile([P128, BL], f32r, tag="h1")
            h2 = mpool.tile([P128, BL], f32r, tag="h2")
            wu = mpool.tile([E, BL], f32, tag="wu")
            for c in range(2):
                cs = slice(c * HB, (c + 1) * HB)
                h1p = psMM.tile([P128, HB], f32, tag="mm1")
                nc.tensor.matmul(h1p[:], W1lo, xT[:, cs], start=True, stop=True)
                nc.scalar.activation(h1[:, cs], h1p[:], Act.Relu, bias=b1,
                                     scale=1.0)
                h2p = psMM.tile([P128, HB], f32, tag="mm2")
                nc.tensor.matmul(h2p[:], W2, h1[:, cs], start=True, stop=True)
                nc.scalar.activation(h2[:, cs], h2p[:], Act.Relu, bias=b2,
                                     scale=1.0)
                lgp = psMM.tile([E, HB], f32, tag="mm3")
                nc.tensor.matmul(lgp[:], W3, h2[:, cs], start=True, stop=True)
                nc.scalar.activation(wu[:, cs], lgp[:], Act.Exp, bias=b3,
                                     scale=1.0)

            # ---- mixture weights back to [batch, expert] layout.
            # Sample b lives at (partition, tile) = (b // 4, b % 4), so tile t
            # takes the stride-4 column slice of wu. ----
            wuv = wu[:].rearrange("e (p t) -> e t p", t=NT)
            wuT = psWU.tile([P128, NT * E], f32, tag="wuT")
            for t in range(NT):
                nc.tensor.transpose(
                    wuT[:, t * E:(t + 1) * E], wuv[:, t, :], idn[:E, :E],
                )
            se4 = spool.tile([P128, NT], f32, tag="se4")
            nc.vector.tensor_reduce(
                se4[:], wuT[:].rearrange("p (t e) -> p t e", t=NT), axis=AxX,
                op=Alu.add,
            )
            r4 = spool.tile([P128, NT], f32, tag="r4")
            nc.vector.reciprocal(r4[:], se4[:])
            wn = spool.tile([P128, NT * E], f32, tag="wn")
            nc.vector.tensor_tensor(
                out=wn[:].rearrange("p (t e) -> p t e", t=NT),
                in0=wuT[:].rearrange("p (t e) -> p t e", t=NT),
                in1=r4[:].unsqueeze(2).broadcast_to([P128, NT, E]),
                op=Alu.mult,
            )

            # ---- combine: per-expert broadcast-TT over all 4 tiles at once
            # (DVE experts 0-4, GpSimd 5-7; Pool has no TensorScalarPtr) ----
            SC4 = spool.tile([P128, NT], f32, tag="SC4")
            comb_all = wpool.tile([P128, NT * N], f32, tag="comb")
            ci_all = wpool.tile([P128, NT * N], f32, tag="ci")
            wnv = wn[:].rearrange("p (t e) -> p t e", t=NT)

            def wb(e):
                return wnv[:, :, e].unsqueeze(2).broadcast_to([P128, NT, N])

            def p3(e):
                return p_sb[e][:].rearrange("p (t n) -> p t n", t=NT)

            bf16 = mybir.dt.bfloat16
            yv = [apool.tile([P128, NT * N], bf16, name=f"y{e}", tag=f"y{e}")
                  for e in range(E)]
            for e in range(6):
                nc.vector.tensor_tensor(
                    yv[e][:].rearrange("p (t n) -> p t n", t=NT), p3(e), wb(e),
                    op=Alu.mult)
            # experts 6,7 multiply on the otherwise-idle ScalarE (per-tile
            # Copy with per-partition scale)
            for e in (6, 7):
                for t in range(NT):
                    sl = slice(t * N, (t + 1) * N)
                    nc.scalar.activation(
                        yv[e][:, sl], p_sb[e][:, sl], Act.Copy,
                        bias=0.0, scale=wn[:, t * E + e:t * E + e + 1],
                    )
            s01 = apool.tile([P128, NT * N], bf16, tag="s01")
            s23 = apool.tile([P128, NT * N], bf16, tag="s23")
            s45 = apool.tile([P128, NT * N], bf16, tag="s45")
            s67 = apool.tile([P128, NT * N], bf16, tag="s67")
            nc.vector.tensor_tensor(s01[:], yv[0][:], yv[1][:], op=Alu.add)
            nc.vector.tensor_tensor(s23[:], yv[2][:], yv[3][:], op=Alu.add)
            nc.vector.tensor_tensor(s45[:], yv[4][:], yv[5][:], op=Alu.add)
            nc.vector.tensor_tensor(s67[:], yv[6][:], yv[7][:], op=Alu.add)
            s03 = apool.tile([P128, NT * N], bf16, tag="s03")
            s47 = apool.tile([P128, NT * N], bf16, tag="s47")
            nc.vector.tensor_tensor(s03[:], s01[:], s23[:], op=Alu.add)
            nc.vector.tensor_tensor(s47[:], s45[:], s67[:], op=Alu.add)
            for t in range(NT):
                sl = slice(t * N, (t + 1) * N)
                nc.vector.tensor_tensor_scan(
                    out=ci_all[:, sl], data0=s03[:, sl],
                    data1=s47[:, sl], initial=0.0,
                    op0=Alu.add, op1=Alu.add,
                )
                nc.vector.tensor_reduce(
                    SC4[:, t:t + 1], ci_all[:, sl], axis=AxX, op=Alu.add)
            nc.vector.tensor_tensor(comb_all[:], s03[:], s47[:], op=Alu.add)
            nc.sync.dma_start(
                out=opdf_d.rearrange("(p t) n -> p t n", p=P128),
                in_=comb_all[:].rearrange("p (t n) -> p t n", t=NT),
            )

            # ---- batched per-sample scalars ----
            # S1 = 1.01*P - 0.01*SC ; Sq = S1 + EPS*sum(s) ; n0 = Sq + 0.25*P
            #    = 1.26*P - 0.01*SC + 0.00505 ; Q = P + 101*EPS
            P4 = ci_all[:].rearrange("p (t n) -> p t n", t=NT)[:, :, N - 1]
            a4 = spool.tile([P128, NT], f32, tag="a4")
            nc.vector.tensor_scalar(a4[:], SC4[:], -0.01, 0.00505, Alu.mult,
                                    Alu.add)
            n04 = spool.tile([P128, NT], f32, tag="n04")
            nc.vector.scalar_tensor_tensor(
                out=n04[:], in0=P4[:], scalar=1.26, in1=a4[:],
                op0=Alu.mult, op1=Alu.add,
            )
            Q4 = spool.tile([P128, NT], f32, tag="Q4")
            nc.vector.tensor_scalar(Q4[:], P4[:], 101.0 * EPS, None, Alu.add)
            Qr4 = spool.tile([P128, NT], f32, tag="Qr4")
            nc.vector.reciprocal(Qr4[:], Q4[:])
            scl4 = spool.tile([P128, NT], f32, tag="scl4")
            nc.vector.tensor_scalar(scl4[:], Qr4[:], -0.5, None, Alu.mult)
            bias4 = spool.tile([P128, NT], f32, tag="bias4")
            nc.vector.tensor_tensor(bias4[:], n04[:], Qr4[:], op=Alu.mult)

            # ---- z = clip(max_k min(z_unc_k, upper_k), 0, 1) per tile ----
            z4c = spool.tile([P128, NT], f32, tag="z4c")
            zu_all = wpool.tile([P128, NT * N], f32, tag="zu")
            N1 = N + 1
            scr_all = wpool.tile([P128, NT * N1], f32, tag="scr")
            nc.vector.memset(scr_all[:], 0.0)
            for t in range(NT):
                sl = slice(t * N, (t + 1) * N)
                nc.scalar.activation(
                    zu_all[:, sl], ci_all[:, sl], Act.Identity,
                    bias=bias4[:, t:t + 1], scale=scl4[:, t:t + 1],
                )
                nc.vector.tensor_tensor(
                    scr_all[:, t * N1:t * N1 + N], zu_all[:, sl], upB,
                    op=Alu.min)
                # the zeroed extra column folds the z >= 0 clip into the max
                nc.vector.tensor_reduce(
                    z4c[:, t:t + 1], scr_all[:, t * N1:(t + 1) * N1],
                    axis=AxX, op=Alu.max)
            nc.sync.dma_start(
                out=oz_d.rearrange("(p t) one -> p t one", p=P128),
                in_=z4c[:].unsqueeze(2),
            )
            err_all = wpool.tile([P128, NT * N], f32, tag="err")
            HT = NT // 2
            oerr_v = oerr_d.rearrange("(p h t) n -> p h t n", p=P128, h=2)
            for hh in range(2):
                hsl = slice(hh * HT * N, (hh + 1) * HT * N)
                nc.vector.tensor_tensor(
                    err_all[:, hsl].rearrange("p (t n) -> p t n", t=HT),
                    sB.unsqueeze(1).broadcast_to([P128, HT, N]),
                    z4c[:, hh * HT:(hh + 1) * HT].unsqueeze(2)
                        .broadcast_to([P128, HT, N]),
                    op=Alu.subtract)
                nc.sync.dma_start(
                    out=oerr_v[:, hh, :, :],
                    in_=err_all[:, hsl].rearrange("p (t n) -> p t n", t=HT),
                )
    nc.compile()
    return nc


def _get_nc():
    if "nc" not in _CACHE:
        _CACHE["nc"] = _build_nc()
    return _CACHE["nc"]


def _run(x, p0, p1, p2, p3, p4, p5, p6, p7, W1, b1, W2, b2, W3, b3, **runkw):
    from concourse.bass_utils import run_bass_kernel_spmd

    nc = _get_nc()
    cst = _const_block(
        np.asarray(W1, np.float32), np.asarray(b1, np.float32),
        np.asarray(W2, np.float32), np.asarray(b2, np.float32),
        np.asarray(W3, np.float32), np.asarray(b3, np.float32),
    )
    ps = [np.ascontiguousarray(np.asarray(p, np.float32))
          for p in (p0, p1, p2, p3, p4, p5, p6, p7)]
    x = np.ascontiguousarray(np.asarray(x, np.float32))
    in_maps = []
    for c in range(NCORES):
        sl = slice(c * BL, (c + 1) * BL)
        m = {"cst": _pack_x(cst, x[sl])}
        for e in range(E):
            m[f"p{e}"] = ps[e][sl]
        in_maps.append(m)
    res = run_bass_kernel_spmd(nc, in_maps, core_ids=list(range(NCORES)), **runkw)
    results = res.results
    pdf = np.concatenate([results[c]["out_pdf"] for c in range(NCORES)], axis=0)
    z = np.concatenate([results[c]["out_z"] for c in range(NCORES)], axis=0)
    err = np.concatenate([results[c]["out_err"] for c in range(NCORES)], axis=0)
    return (pdf, z, err), res


def kernel(x, p0, p1, p2, p3, p4, p5, p6, p7, W1, b1, W2, b2, W3, b3):
    out, _ = _run(x, p0, p1, p2, p3, p4, p5, p6, p7,
                  W1, b1, W2, b2, W3, b3)
    return out


# revision 43
# speedup vs baseline: 1.0308x; 1.0007x over previous
"""Trainium2 Bass kernel for AdaptiveLinearPoolNewsvendorLayer.

Computes, for B=4096 samples across 8 NeuronCores (data parallel, 512/core):
  w        = softmax(MLP(x))                  # [B, 8] mixture weights
  pdf      = sum_e w[:, e] * p_e              # [B, 101] convex combination
  z        = argmin_z of the per-sample newsvendor QP (exact, convex)
  error    = SUPPORT - z

The QP solve is reformulated analytically: with TAU=RA=0.5, EPS=1e-4 the
piecewise-quadratic objective's stationary point on interval k is
  z_unc_k = (Sq + 0.25*P - 0.5*C_k) / Q
with C_k the pdf mass strictly below the interval, P = sum(pdf),
Sq = sum((pdf+EPS)*s), Q = P + 101*EPS.  Since z_unc is non-increasing in k
while the intervals increase, the unique minimizer is
  z = clip(max_k min(z_unc_k, upper_k), 0, 1)
which needs only a cumsum + one fused min/max-reduce per sample instead of
the brute-force 102x101 objective evaluation.
"""

import numpy as np

B, D, H, E, N = 4096, 64, 128, 8, 101
NCORES = 8
BL = B // NCORES          # 512 rows per core
P128 = 128
NT = BL // P128           # 4 partition-tiles per core
EPS = 1e-4
USE_TTR = False

# constant-block column offsets (one [128, CST_COLS] f32 DRAM tensor per core;
# split into a critical DMA [0,_SB_C) = weights+biases+x and a tail DMA)
_W1_C = 0                  # cols [0,128)  W1 [64,128] lhsT, duplicated rows 64:128
_W2_C = 128                # cols [128,256)            : W2 [128,128]
_W3_C = 256                # cols [256,264)            : W3 [128,8]
_B1_C = 264                # col 264                   : b1 [128]
_B2_C = 265                # col 265                   : b2 [128]
_B3_C = 266                # col 266 rows 0:8          : b3 [8]
_X_C = 267                 # cols [267,779) rows 0:64  : xT [64, 512]
_SB_C = 779                # cols [779,880)            : support grid, replicated
_UP_C = 880                # cols [880,981)            : upper_k (k=1..101), replicated
_ID_C = 981                # cols [981,1109)           : I_128
CST_COLS = 1109

_CACHE = {}


def _const_block(W1, b1, W2, b2, W3, b3):
    cst = np.zeros((P128, CST_COLS), np.float32)
    cst[:D, _W1_C:_W1_C + H] = W1
    cst[D:, _W1_C:_W1_C + H] = W1
    cst[:, _W2_C:_W2_C + H] = W2
    cst[:, _W3_C:_W3_C + E] = W3
    cst[:, _B1_C] = b1
    cst[:, _B2_C] = b2
    cst[:E, _B3_C] = b3
    S = np.linspace(0.0, 1.0, N, dtype=np.float32)
    cst[:, _SB_C:_SB_C + N] = S[None, :]
    upper = np.concatenate([S[1:], S[-1:]])
    cst[:, _UP_C:_UP_C + N] = upper[None, :]
    cst[:, _ID_C:_ID_C + P128] = np.eye(P128, dtype=np.float32)
    return cst


def _pack_x(cst, x_core):
    # xT [64, 512] packed as two column-halves stacked: rows 0:64 hold
    # xT[:, 0:256], rows 64:128 hold xT[:, 256:512] (zero waste, and W1 is
    # duplicated into rows 64:128 so layer 1 runs as two matmuls).
    out = cst.copy()
    out[:D, _X_C:_X_C + BL] = x_core.T
    return out


def _build_nc():
    import concourse.bass as bass
    import concourse.bacc as bacc
    import concourse.tile as tile
    from concourse import mybir

    f32 = mybir.dt.float32
    f32r = mybir.dt.float32r
    Alu = mybir.AluOpType
    Act = mybir.ActivationFunctionType
    AxX = mybir.AxisListType.X

    nc = bacc.Bacc()

    p_d = [nc.declare_dram_parameter(f"p{e}", [BL, N], f32, isOutput=False)
           for e in range(E)]
    cst_d = nc.declare_dram_parameter("cst", [P128, CST_COLS], f32,
                                      isOutput=False)
    opdf_d = nc.declare_dram_parameter("out_pdf", [BL, N], f32, isOutput=True)
    oz_d = nc.declare_dram_parameter("out_z", [BL, 1], f32, isOutput=True)
    oerr_d = nc.declare_dram_parameter("out_err", [BL, N], f32, isOutput=True)

    NDV = 4   # experts combined on DVE; the rest go to GpSimd

    with tile.TileContext(nc) as tc:
        with (
            tc.tile_pool(name="consts", bufs=1) as cpool,
            tc.tile_pool(name="mlp", bufs=1) as mpool,
            tc.tile_pool(name="pin", bufs=1) as ppool,
            tc.tile_pool(name="work", bufs=1) as wpool,
            tc.tile_pool(name="acc", bufs=2) as apool,
            tc.tile_pool(name="stats", bufs=1) as spool,
            tc.tile_pool(name="ps_mm", bufs=2, space="PSUM") as psMM,
            tc.tile_pool(name="ps_wu", bufs=1, space="PSUM") as psWU,
            tc.tile_pool(name="ps_wm", bufs=1, space="PSUM") as psWM,
        ):
            # ---- PE p-state warmup: harmless matmuls on a zeroed tile while
            # the input DMAs are in flight (PE ramps to full clock after ~3us
            # of continuous work) ----
            zw = cpool.tile([P128, P128], f32, tag="zw")
            nc.vector.memset(zw[:], 0.0)
            wps = psWM.tile([P128, P128], f32, tag="wps")
            bf16 = mybir.dt.bfloat16
            zwb = zw[:].bitcast(bf16)[:, :P128]
            for _ in range(16):
                nc.tensor.matmul(wps[:], zwb, zwb, start=True, stop=True)
            zwa = cpool.tile([P128, P128], f32, tag="zwa")
            for _ in range(4):
                nc.scalar.activation(zwa[:], zw[:].bitcast(f32), Act.Exp,
                                     bias=0.0, scale=1.0)

            cstr = cpool.tile([P128, CST_COLS], f32r, tag="cst")
            nc.sync.dma_start(out=cstr[:, :_SB_C],
                              in_=cst_d[:, :_SB_C].bitcast(f32r))
            nc.sync.dma_start(out=cstr[:, _SB_C:],
                              in_=cst_d[:, _SB_C:].bitcast(f32r))
            cst = cstr[:].bitcast(f32)
            W1lo = cstr[:D, _W1_C:_W1_C + H]
            W2 = cstr[:, _W2_C:_W2_C + H]
            W3 = cstr[:, _W3_C:_W3_C + E]
            xT = cstr[:D, _X_C:_X_C + BL]
            b1 = cst[:, _B1_C:_B1_C + 1]
            b2 = cst[:, _B2_C:_B2_C + 1]
            b3 = cst[:E, _B3_C:_B3_C + 1]
            sB = cst[:, _SB_C:_SB_C + N]
            upB = cst[:, _UP_C:_UP_C + N]
            idn = cst[:, _ID_C:_ID_C + P128]

            p_sb = []
            for e in range(E):
                pt = ppool.tile([P128, NT * N], f32, tag=f"p{e}")
                nc.sync.dma_start(
                    out=pt[:].rearrange("p (t n) -> p t n", t=NT),
                    in_=p_d[e].rearrange("(p t) n -> p t n", p=P128),
                )
                p_sb.append(pt)

            # ---- MLP in [feature, batch] layout, pipelined in two
            # 256-column chunks (separate PSUM banks so ACT overlaps PE) ----
            HB = BL // 2
            h1 = mpool.tile([P128, BL], f32r, tag="h1")
            h2 = mpool.tile([P128, BL], f32r, tag="h2")
            wu = mpool.tile([E, BL], f32, tag="wu")
            for c in range(2):
                cs = slice(c * HB, (c + 1) * HB)
                h1p = psMM.tile([P128, HB], f32, tag="mm1")
                nc.tensor.matmul(h1p[:], W1lo, xT[:, cs], start=True, stop=True)
                if c == 0:
                    # chunk-a relus on DVE so both chunks' activations overlap
                    nc.vector.tensor_scalar(h1[:, cs], h1p[:], b1, 0.0,
                                            Alu.add, Alu.max)
                else:
                    nc.scalar.activation(h1[:, cs], h1p[:], Act.Relu, bias=b1,
                                         scale=1.0)
                h2p = psMM.tile([P128, HB], f32, tag="mm2")
                nc.tensor.matmul(h2p[:], W2, h1[:, cs], start=True, stop=True)
                if c == 0:
                    nc.vector.tensor_scalar(h2[:, cs], h2p[:], b2, 0.0,
                                            Alu.add, Alu.max)
                else:
                    nc.scalar.activation(h2[:, cs], h2p[:], Act.Relu, bias=b2,
                                         scale=1.0)
                lgp = psMM.tile([E, HB], f32, tag="mm3")
                nc.tensor.matmul(lgp[:], W3, h2[:, cs], start=True, stop=True)
                nc.scalar.activation(wu[:, cs], lgp[:], Act.Exp, bias=b3,
                                     scale=1.0)

            # ---- mixture weights back to [batch, expert] layout.
            # Sample b lives at (partition, tile) = (b // 4, b % 4), so tile t
            # takes the stride-4 column slice of wu. ----
            wuv = wu[:].rearrange("e (p t) -> e t p", t=NT)
            wuT = psWU.tile([P128, NT * E], f32, tag="wuT")
            for t in range(NT):
                nc.tensor.transpose(
                    wuT[:, t * E:(t + 1) * E], wuv[:, t, :], idn[:E, :E],
                )
            se4 = spool.tile([P128, NT], f32, tag="se4")
            nc.vector.tensor_reduce(
                se4[:], wuT[:].rearrange("p (t e) -> p t e", t=NT), axis=AxX,
                op=Alu.add,
            )
            r4 = spool.tile([P128, NT], f32, tag="r4")
            nc.vector.reciprocal(r4[:], se4[:])
            wn = spool.tile([P128, NT * E], f32, tag="wn")
            nc.vector.tensor_tensor(
                out=wn[:].rearrange("p (t e) -> p t e", t=NT),
                in0=wuT[:].rearrange("p (t e) -> p t e", t=NT),
                in1=r4[:].unsqueeze(2).broadcast_to([P128, NT, E]),
                op=Alu.mult,
            )

            # ---- combine: per-expert broadcast-TT over all 4 tiles at once
            # (DVE experts 0-4, GpSimd 5-7; Pool has no TensorScalarPtr) ----
            SC4 = spool.tile([P128, NT], f32, tag="SC4")
            comb_all = wpool.tile([P128, NT * N], f32, tag="comb")
            ci_all = wpool.tile([P128, NT * N], f32, tag="ci")
            wnv = wn[:].rearrange("p (t e) -> p t e", t=NT)

            def wb(e):
                return wnv[:, :, e].unsqueeze(2).broadcast_to([P128, NT, N])

            def p3(e):
                return p_sb[e][:].rearrange("p (t n) -> p t n", t=NT)

            bf16 = mybir.dt.bfloat16
            yv = [apool.tile([P128, NT * N], bf16, name=f"y{e}", tag=f"y{e}")
                  for e in range(E)]
            for e in range(6):
                nc.vector.tensor_tensor(
                    yv[e][:].rearrange("p (t n) -> p t n", t=NT), p3(e), wb(e),
                    op=Alu.mult)
            # experts 6,7 multiply on the otherwise-idle ScalarE (per-tile
            # Copy with per-partition scale)
            for e in (6, 7):
                for t in range(NT):
                    sl = slice(t * N, (t + 1) * N)
                    nc.scalar.activation(
                        yv[e][:, sl], p_sb[e][:, sl], Act.Copy,
                        bias=0.0, scale=wn[:, t * E + e:t * E + e + 1],
                    )
            s01 = apool.tile([P128, NT * N], bf16, tag="s01")
            s23 = apool.tile([P128, NT * N], bf16, tag="s23")
            s45 = apool.tile([P128, NT * N], bf16, tag="s45")
            s67 = apool.tile([P128, NT * N], bf16, tag="s67")
            nc.vector.tensor_tensor(s01[:], yv[0][:], yv[1][:], op=Alu.add)
            nc.vector.tensor_tensor(s23[:], yv[2][:], yv[3][:], op=Alu.add)
            nc.vector.tensor_tensor(s45[:], yv[4][:], yv[5][:], op=Alu.add)
            nc.vector.tensor_tensor(s67[:], yv[6][:], yv[7][:], op=Alu.add)
            s03 = apool.tile([P128, NT * N], bf16, tag="s03")
            s47 = apool.tile([P128, NT * N], bf16, tag="s47")
            nc.vector.tensor_tensor(s03[:], s01[:], s23[:], op=Alu.add)
            nc.vector.tensor_tensor(s47[:], s45[:], s67[:], op=Alu.add)
            for t in range(NT):
                sl = slice(t * N, (t + 1) * N)
                nc.vector.tensor_tensor_scan(
                    out=ci_all[:, sl], data0=s03[:, sl],
                    data1=s47[:, sl], initial=0.0,
                    op0=Alu.add, op1=Alu.add,
                )
                nc.vector.tensor_reduce(
                    SC4[:, t:t + 1], ci_all[:, sl], axis=AxX, op=Alu.add)
            nc.vector.tensor_tensor(comb_all[:], s03[:], s47[:], op=Alu.add)
            nc.sync.dma_start(
                out=opdf_d.rearrange("(p t) n -> p t n", p=P128),
                in_=comb_all[:].rearrange("p (t n) -> p t n", t=NT),
            )

            # ---- batched per-sample scalars ----
            # S1 = 1.01*P - 0.01*SC ; Sq = S1 + EPS*sum(s) ; n0 = Sq + 0.25*P
            #    = 1.26*P - 0.01*SC + 0.00505 ; Q = P + 101*EPS
            P4 = ci_all[:].rearrange("p (t n) -> p t n", t=NT)[:, :, N - 1]
            a4 = spool.tile([P128, NT], f32, tag="a4")
            nc.vector.tensor_scalar(a4[:], SC4[:], -0.01, 0.00505, Alu.mult,
                                    Alu.add)
            n04 = spool.tile([P128, NT], f32, tag="n04")
            nc.vector.scalar_tensor_tensor(
                out=n04[:], in0=P4[:], scalar=1.26, in1=a4[:],
                op0=Alu.mult, op1=Alu.add,
            )
            Q4 = spool.tile([P128, NT], f32, tag="Q4")
            nc.vector.tensor_scalar(Q4[:], P4[:], 101.0 * EPS, None, Alu.add)
            Qr4 = spool.tile([P128, NT], f32, tag="Qr4")
            nc.vector.reciprocal(Qr4[:], Q4[:])
            scl4 = spool.tile([P128, NT], f32, tag="scl4")
            nc.vector.tensor_scalar(scl4[:], Qr4[:], -0.5, None, Alu.mult)
            bias4 = spool.tile([P128, NT], f32, tag="bias4")
            nc.vector.tensor_tensor(bias4[:], n04[:], Qr4[:], op=Alu.mult)

            # ---- z = clip(max_k min(z_unc_k, upper_k), 0, 1) per tile ----
            z4c = spool.tile([P128, NT], f32, tag="z4c")
            zu_all = wpool.tile([P128, NT * N], f32, tag="zu")
            N1 = N + 1
            scr_all = wpool.tile([P128, NT * N1], f32, tag="scr")
            nc.vector.memset(scr_all[:], 0.0)
            for t in range(NT):
                sl = slice(t * N, (t + 1) * N)
                nc.scalar.activation(
                    zu_all[:, sl], ci_all[:, sl], Act.Identity,
                    bias=bias4[:, t:t + 1], scale=scl4[:, t:t + 1],
                )
                nc.vector.tensor_tensor(
                    scr_all[:, t * N1:t * N1 + N], zu_all[:, sl], upB,
                    op=Alu.min)
                # the zeroed extra column folds the z >= 0 clip into the max
                nc.vector.tensor_reduce(
                    z4c[:, t:t + 1], scr_all[:, t * N1:(t + 1) * N1],
                    axis=AxX, op=Alu.max)
            nc.sync.dma_start(
                out=oz_d.rearrange("(p t) one -> p t one", p=P128),
                in_=z4c[:].unsqueeze(2),
            )
            err_all = wpool.tile([P128, NT * N], f32, tag="err")
            HT = NT // 2
            oerr_v = oerr_d.rearrange("(p h t) n -> p h t n", p=P128, h=2)
            for hh in range(2):
                hsl = slice(hh * HT * N, (hh + 1) * HT * N)
                nc.vector.tensor_tensor(
                    err_all[:, hsl].rearrange("p (t n) -> p t n", t=HT),
                    sB.unsqueeze(1).broadcast_to([P128, HT, N]),
                    z4c[:, hh * HT:(hh + 1) * HT].unsqueeze(2)
                        .broadcast_to([P128, HT, N]),
                    op=Alu.subtract)
                nc.sync.dma_start(
                    out=oerr_v[:, hh, :, :],
                    in_=err_all[:, hsl].rearrange("p (t n) -> p t n", t=HT),
                )
    nc.compile()
    return nc


def _get_nc():
    if "nc" not in _CACHE:
        _CACHE["nc"] = _build_nc()
    return _CACHE["nc"]


def _run(x, p0, p1, p2, p3, p4, p5, p6, p7, W1, b1, W2, b2, W3, b3, **runkw):
    from concourse.bass_utils import run_bass_kernel_spmd

    nc = _get_nc()
    cst = _const_block(
        np.asarray(W1, np.float32), np.asarray(b1, np.float32),
        np.asarray(W2, np.float32), np.asarray(b2, np.float32),
        np.asarray(W3, np.float32), np.asarray(b3, np.float32),
    )
    ps = [np.ascontiguousarray(np.asarray(p, np.float32))
          for p in (p0, p1, p2, p3, p4, p5, p6, p7)]
    x = np.ascontiguousarray(np.asarray(x, np.float32))
    in_maps = []
    for c in range(NCORES):
        sl = slice(c * BL, (c + 1) * BL)
        m = {"cst": _pack_x(cst, x[sl])}
        for e in range(E):
            m[f"p{e}"] = ps[e][sl]
        in_maps.append(m)
    res = run_bass_kernel_spmd(nc, in_maps, core_ids=list(range(NCORES)), **runkw)
    results = res.results
    pdf = np.concatenate([results[c]["out_pdf"] for c in range(NCORES)], axis=0)
    z = np.concatenate([results[c]["out_z"] for c in range(NCORES)], axis=0)
    err = np.concatenate([results[c]["out_err"] for c in range(NCORES)], axis=0)
    return (pdf, z, err), res


def kernel(x, p0, p1, p2, p3, p4, p5, p6, p7, W1, b1, W2, b2, W3, b3):
    out, _ = _run(x, p0, p1, p2, p3, p4, p5, p6, p7,
                  W1, b1, W2, b2, W3, b3)
    return out
